# revision 1
# baseline (speedup 1.0000x reference)
"""MoE (gating + 8 experts, BN-folded) Trainium2 Bass kernel.

Contract: kernel(**inputs) takes the FULL unsharded inputs (numpy, keyed as in
setup_inputs()) and returns the FULL [65536, 1] float32 output.

Strategy:
  * Data-parallel over 8 NeuronCores: batch 65536 -> 8192 rows per core.
  * All BatchNorms are eval-mode affine maps -> folded into the adjacent
    Linear weights/biases on the host (cheap: params < 2 MB).
  * Activations live on-chip as [features(partitions), batch(free)] so the
    Linear chain needs no transposes; x is transposed host-side per shard.
  * All matmuls run as float32r (fp32 with 12-bit mantissa, fast PE mode).
  * Gating: softmax normalization is deferred - unnormalized exp(logits) are
    used as gate weights; the kernel exports the raw combined row and the
    softmax row-sum, and the host does y = raw/sum + ob.
  * Experts are processed in pairs; layer-2 uses block-diagonal [256->128]
    weights so two experts' H2=64 outputs stack into one 128-partition tile.
  * Combine: gate rows broadcast across partitions with a K=8 selector
    matmul; one DVE multiply per pair; output projection oW accumulates the 4
    pair products into one PSUM [1,512] row.
  * x loads issue on the Sync queue, output stores on the GpSimd queue so
    stores never head-of-line-block the next tile's loads.
"""

import numpy as np
import ml_dtypes

EPS = 1e-5
B, D, E, G, H0, H1, H2 = 65536, 256, 8, 128, 256, 128, 64
NCORES = 8
NB = B // NCORES          # rows per core
TB = 512                  # batch tile (matmul free dim / PSUM bank)
NT = NB // TB             # batch tiles per core
KD = D // 128             # k-chunks over D
NPAIR = E // 2


def _round_f32r(a):
    """Round float32 -> fp32r encoding (fp32 with 12 explicit mantissa bits,
    round-to-nearest-even). Matches walrus fp32_to_fp32r."""
    b = np.ascontiguousarray(a, dtype=np.float32).view(np.uint32).copy()
    low = b & np.uint32(0xFFF)
    b &= np.uint32(0xFFFFF000)
    rup = (low > 0x800) | ((low == 0x800) & (((b >> np.uint32(12)) & np.uint32(1)) == 1))
    b[rup] += np.uint32(0x1000)
    return b.view(np.float32)


def _fold_params(inputs):
    """Fold the four BatchNorms into the adjacent Linears. float64 math."""
    f = {k: np.asarray(v, dtype=np.float64) for k, v in inputs.items()}

    s_in = f["in_g"] / np.sqrt(f["in_v"] + EPS)            # [D]
    t_in = f["in_b"] - f["in_m"] * s_in                    # [D]

    # gating L1 (+input BN folded in)
    a_g = f["g_g"] / np.sqrt(f["g_v"] + EPS)               # [G]
    w1 = f["gW1"] * a_g[None, :]                           # [D,G]
    W1f = s_in[:, None] * w1
    b1f = t_in @ w1 + (f["gb1"] - f["g_m"]) * a_g + f["g_b"]

    # expert L0 (+input BN)
    a0 = f["e0g"] / np.sqrt(f["e0v"] + EPS)                # [E,H0]
    w0 = f["eW0"] * a0[:, None, :]                         # [E,D,H0]
    W0f = s_in[None, :, None] * w0
    b0f = np.einsum("d,edo->eo", t_in, w0) + (f["eb0"] - f["e0m"]) * a0 + f["e0b"]

    a1 = f["e1g"] / np.sqrt(f["e1v"] + EPS)
    W1ef = f["eW1"] * a1[:, None, :]                       # [E,H0,H1]
    b1ef = (f["eb1"] - f["e1m"]) * a1 + f["e1b"]

    a2 = f["e2g"] / np.sqrt(f["e2v"] + EPS)
    W2f = f["eW2"] * a2[:, None, :]                        # [E,H1,H2]
    b2f = (f["eb2"] - f["e2m"]) * a2 + f["e2b"]

    g = lambda a: np.ascontiguousarray(a, dtype=np.float32)

    dev = {}
    dev["WG1"] = g(W1f.reshape(KD, 128, G).transpose(1, 0, 2))          # [128,KD,G]
    dev["BG1"] = g(b1f[:, None])                                        # [G,1]
    dev["WG2"] = g(f["gW2"])                                            # [G,E]
    dev["BG2"] = g(f["gb2"][:, None])                                   # [E,1]
    dev["WE0"] = g(W0f.reshape(E, KD, 128, 2, 128).transpose(2, 0, 1, 3, 4))  # [128,E,KD,2,128]
    dev["BE0"] = g(b0f.reshape(E, 2, 128).transpose(2, 0, 1))           # [128,E,2]
    dev["WE1"] = g(W1ef.reshape(E, 2, 128, H1).transpose(2, 0, 1, 3))   # [128,E,2,H1]
    dev["BE1"] = g(b1ef.T)                                              # [H1,E]
    WE2 = np.zeros((128, NPAIR, 2, 128), dtype=np.float64)
    BE2 = np.zeros((128, NPAIR), dtype=np.float64)
    for j in range(NPAIR):
        WE2[:, j, 0, 0:64] = W2f[2 * j]                                 # K rows = h1 of expert 2j
        WE2[:, j, 1, 64:128] = W2f[2 * j + 1]
        BE2[0:64, j] = b2f[2 * j]
        BE2[64:128, j] = b2f[2 * j + 1]
    dev["WE2"] = g(WE2)
    dev["BE2"] = g(BE2)
    ow = f["oW"][:, 0]                                                  # [H2]
    dev["OWP"] = g(np.concatenate([ow, ow])[:, None])                   # [128,1]
    SP = np.zeros((E, NPAIR, 128))
    for j in range(NPAIR):
        SP[2 * j, j, 0:64] = 1.0
        SP[2 * j + 1, j, 64:128] = 1.0
    dev["SP"] = g(SP)
    dev["ONES8"] = g(np.ones((E, E)))
    for name in F32R_INPUTS:
        dev[name] = _round_f32r(dev[name])
    for name in BF16_INPUTS:
        dev[name] = dev[name].astype(ml_dtypes.bfloat16)
    ob = float(f["ob"][0])
    return dev, ob


# DMA-fed matmul operands by mode. "mixed" keeps the gating/combine path in
# fp32r and runs the expert chain bf16; "bf16" runs every matmul bf16;
# "f32r" runs everything fp32r (most accurate, ~2x PE cost).
MODE = "bf16"
_ALL_MM = ("WG1", "WG2", "OWP", "SP", "ONES8", "WE0", "WE1", "WE2")
if MODE == "f32r":
    F32R_INPUTS, BF16_INPUTS = _ALL_MM, ()
elif MODE == "bf16":
    F32R_INPUTS, BF16_INPUTS = (), _ALL_MM
else:
    F32R_INPUTS = ("WG1", "WG2", "OWP", "SP", "ONES8")
    BF16_INPUTS = ("WE0", "WE1", "WE2")


def _build_program():
    import concourse.bass as bass
    import concourse.mybir as mybir
    import concourse.tile as tile
    from concourse import bacc

    f32 = mybir.dt.float32
    f32r = mybir.dt.float32r
    bf16 = mybir.dt.bfloat16
    Relu = mybir.ActivationFunctionType.Relu
    Exp = mybir.ActivationFunctionType.Exp
    add = mybir.AluOpType.add
    amax = mybir.AluOpType.max

    def dtype_of(name):
        if name in F32R_INPUTS:
            return f32r
        if name in BF16_INPUTS:
            return bf16
        return f32

    g_dt = f32r if MODE in ("f32r", "mixed") else bf16   # gating/combine dtype
    e_dt = bf16 if MODE in ("bf16", "mixed") else f32r   # expert-chain dtype

    nc = bacc.Bacc("TRN2", target_bir_lowering=False, debug=False)

    xT = nc.dram_tensor("xT", [D, NB], g_dt, kind="ExternalInput").ap()
    xTb = (nc.dram_tensor("xTb", [D, NB], e_dt, kind="ExternalInput").ap()
           if g_dt != e_dt else None)
    yraw = nc.dram_tensor("yraw", [1, NB], f32, kind="ExternalOutput").ap()
    rsum = nc.dram_tensor("rsum", [1, NB], f32, kind="ExternalOutput").ap()
    d_in = {}
    for name, shape in [
        ("WG1", [128, KD, G]), ("BG1", [G, 1]), ("WG2", [G, E]), ("BG2", [E, 1]),
        ("WE0", [128, E, KD, 2, 128]), ("BE0", [128, E, 2]),
        ("WE1", [128, E, 2, H1]), ("BE1", [H1, E]),
        ("WE2", [128, NPAIR, 2, 128]), ("BE2", [128, NPAIR]),
        ("OWP", [128, 1]), ("SP", [E, NPAIR, 128]), ("ONES8", [E, E]),
    ]:
        d_in[name] = nc.dram_tensor(name, shape, dtype_of(name), kind="ExternalInput").ap()

    with tile.TileContext(nc) as tc:
        with (
            tc.tile_pool(name="consts", bufs=1) as consts,
            tc.tile_pool(name="xt", bufs=4) as xtp,
            tc.tile_pool(name="act", bufs=4) as actp,
            tc.tile_pool(name="h1p", bufs=6) as h1p,
            tc.tile_pool(name="small", bufs=4) as smallp,
            tc.tile_pool(name="pmm", bufs=5, space="PSUM") as pmm,
            tc.tile_pool(name="psm", bufs=2, space="PSUM") as psm,
            tc.tile_pool(name="pgbc", bufs=1, space="PSUM") as pgbc,
        ):
            W = {}
            for name, ap in d_in.items():
                W[name] = consts.tile(list(ap.shape), dtype_of(name), tag=name, name=name)
                if name in ("WE0", "WE1", "WE2"):
                    # split per expert so tile-0 compute starts as soon as
                    # the first expert's weights land (usage-order loads)
                    continue
                nc.gpsimd.dma_start(W[name][:], ap[:])
            for e in range(E):
                nc.gpsimd.dma_start(W["WE0"][:, e], d_in["WE0"][:, e])
                nc.gpsimd.dma_start(W["WE1"][:, e], d_in["WE1"][:, e])
                if e < NPAIR:
                    nc.gpsimd.dma_start(W["WE2"][:, e], d_in["WE2"][:, e])

            for t in range(NT):
                bs = t * TB
                xt = xtp.tile([128, KD, TB], g_dt, tag="xt")
                for c in range(KD):
                    nc.sync.dma_start(xt[:, c, :], xT[c * 128:(c + 1) * 128, bs:bs + TB])
                if g_dt == e_dt:
                    xtb = xt
                else:
                    xtb = xtp.tile([128, KD, TB], e_dt, tag="xtb")
                    for c in range(KD):
                        nc.sync.dma_start(xtb[:, c, :],
                                          xTb[c * 128:(c + 1) * 128, bs:bs + TB])

                # ---- gating ----
                ps_g = pmm.tile([128, TB], f32, tag="mm")
                for c in range(KD):
                    nc.tensor.matmul(ps_g[:], W["WG1"][:, c, :], xt[:, c, :],
                                     start=(c == 0), stop=(c == KD - 1))
                gh = actp.tile([128, TB], g_dt, tag="gh")
                nc.scalar.activation(gh[:], ps_g[:], Relu, bias=W["BG1"][:, 0:1])

                ps_l = psm.tile([E, TB], f32, tag="small")
                nc.tensor.matmul(ps_l[:], W["WG2"][:], gh[:], start=True, stop=True)
                expg = smallp.tile([E, TB], g_dt, tag="expg")
                nc.scalar.activation(expg[:], ps_l[:], Exp, bias=W["BG2"][:, 0:1])

                ps_s = psm.tile([E, TB], f32, tag="small")
                nc.tensor.matmul(ps_s[:], W["ONES8"][:], expg[:], start=True, stop=True)
                srow = smallp.tile([1, TB], f32, tag="srow")
                nc.scalar.copy(srow[:], ps_s[0:1, :])
                nc.gpsimd.dma_start(rsum[0:1, bs:bs + TB], srow[:])

                ps_out = psm.tile([1, TB], f32, tag="small", name="ps_out")

                for j in range(NPAIR):
                    h1t = [None, None]
                    for i in (0, 1):
                        e = 2 * j + i
                        # ---- expert L0: D=256 -> H0=256 (2 K-chunks x 2 M-chunks)
                        ps0 = [pmm.tile([128, TB], f32, tag="mm", name=f"ps0_{i}")
                               for i in range(2)]
                        for mc in range(2):
                            for c in range(KD):
                                nc.tensor.matmul(ps0[mc][:], W["WE0"][:, e, c, mc, :],
                                                 xtb[:, c, :],
                                                 start=(c == 0), stop=(c == KD - 1))
                        h0 = actp.tile([128, 2, TB], e_dt, tag="h0")
                        nc.scalar.activation(h0[:, 0, :], ps0[0][:], Relu,
                                             bias=W["BE0"][:, e, 0:1])
                        nc.vector.tensor_scalar(out=h0[:, 1, :], in0=ps0[1][:],
                                                scalar1=W["BE0"][:, e, 1:2], scalar2=0.0,
                                                op0=add, op1=amax)
                        # ---- expert L1: H0=256 -> H1=128
                        ps1 = pmm.tile([128, TB], f32, tag="mm")
                        for c in range(2):
                            nc.tensor.matmul(ps1[:], W["WE1"][:, e, c, :], h0[:, c, :],
                                             start=(c == 0), stop=(c == 1))
                        h1t[i] = h1p.tile([128, TB], e_dt, tag="h1", name=f"h1_{i}")
                        if i == 0:
                            nc.scalar.activation(h1t[i][:], ps1[:], Relu,
                                                 bias=W["BE1"][:, e:e + 1])
                        else:
                            nc.vector.tensor_scalar(out=h1t[i][:], in0=ps1[:],
                                                    scalar1=W["BE1"][:, e:e + 1],
                                                    scalar2=0.0, op0=add, op1=amax)
                    # ---- expert L2 (paired, block-diagonal): 2x(H1->H2) -> [128,TB]
                    ps2 = pmm.tile([128, TB], f32, tag="mm")
                    for c in range(2):
                        nc.tensor.matmul(ps2[:], W["WE2"][:, j, c, :], h1t[c][:],
                                         start=(c == 0), stop=(c == 1))
                    h2 = actp.tile([128, TB], f32, tag="h2")
                    if j % 2 == 0:
                        nc.scalar.activation(h2[:], ps2[:], Relu, bias=W["BE2"][:, j:j + 1])
                    else:
                        nc.vector.tensor_scalar(out=h2[:], in0=ps2[:],
                                                scalar1=W["BE2"][:, j:j + 1], scalar2=0.0,
                                                op0=add, op1=amax)
                    # ---- gate broadcast for this pair + weighted product
                    gbc = pgbc.tile([128, TB], f32, tag="gbc")
                    nc.tensor.matmul(gbc[:], W["SP"][:, j, :], expg[:],
                                     start=True, stop=True)
                    pw = h1p.tile([128, TB], g_dt, tag="pw")
                    nc.vector.tensor_mul(pw[:], h2[:], gbc[:])
                    # ---- output projection accumulates over pairs
                    nc.tensor.matmul(ps_out[:], W["OWP"][:], pw[:],
                                     start=(j == 0), stop=(j == NPAIR - 1))

                # ---- export raw combined row (host divides by rsum, adds ob)
                orow = smallp.tile([1, TB], f32, tag="orow")
                nc.scalar.copy(orow[:], ps_out[0:1, :])
                nc.gpsimd.dma_start(yraw[0:1, bs:bs + TB], orow[:])

    nc.compile()
    return nc


_CACHE = {}


def _get_program():
    if "nc" not in _CACHE:
        _CACHE["nc"] = _build_program()
    return _CACHE["nc"]


def _run(inputs, trace=False):
    from concourse.bass_utils import run_bass_kernel_spmd

    x = np.ascontiguousarray(np.asarray(inputs["x"], dtype=np.float32))
    dev, ob = _fold_params(inputs)
    nc = _get_program()

    g_np = _round_f32r if MODE in ("f32r", "mixed") else (lambda a: a.astype(ml_dtypes.bfloat16))
    e_np = (lambda a: a.astype(ml_dtypes.bfloat16)) if MODE in ("bf16", "mixed") else _round_f32r
    in_maps = []
    for c in range(NCORES):
        m = dict(dev)
        xs = np.ascontiguousarray(x[c * NB:(c + 1) * NB, :].T)
        m["xT"] = g_np(xs)
        if MODE == "mixed":
            m["xTb"] = e_np(xs)
        in_maps.append(m)

    kwargs = {}
    if trace:
        kwargs = dict(trace=True, trace_cores=[0])
    res = run_bass_kernel_spmd(nc, in_maps, core_ids=list(range(NCORES)), **kwargs)
    yr = np.concatenate([res.results[c]["yraw"].reshape(-1) for c in range(NCORES)])
    rs = np.concatenate([res.results[c]["rsum"].reshape(-1) for c in range(NCORES)])
    out = (yr.astype(np.float64) / rs.astype(np.float64)) + ob
    return out.astype(np.float32)[:, None], res


def kernel(**inputs):
    out, _ = _run(inputs, trace=False)
    return out


def kernel_traced(**inputs):
    return _run(inputs, trace=True)



# revision 2
# speedup vs baseline: 1.1689x; 1.1689x over previous
"""MoE (gating + 8 experts, BN-folded) Trainium2 Bass kernel.

Contract: kernel(**inputs) takes the FULL unsharded inputs (numpy, keyed as in
setup_inputs()) and returns the FULL [65536, 1] float32 output.

Strategy (v2):
  * Data-parallel over 8 NeuronCores: batch 65536 -> 8192 rows per core.
  * All BatchNorms are eval-mode affine maps -> folded into the adjacent
    Linear weights/biases on the host.
  * Activations live on-chip as [features(partitions), batch(free)]; x is
    transposed host-side per shard. All matmuls bf16.
  * The device computes only the dense GEMM chain: gating L1 (gh) and the
    expert stack L0/L1/L2 (h2). gh and h2 are DMA-exported; the host does the
    tiny tail: gating L2 (gh @ gW2), softmax weights, s = ow . h2, and the
    gate-weighted combine. This removes all partition-reduction matmuls
    (SP/OWP/ONES8) from the Tensor engine stream.
  * Expert L2 runs as two col-tiled M=64 matmuls per expert pair
    (tile_position (0,0)/(0,64)) which execute concurrently in the PE array.
  * Phase-major loop per batch tile (all L0s -> all L1s -> all L2s) gives
    PSUM evictions slack so matmuls don't stall on freshly-written operands.
"""

import numpy as np
import ml_dtypes

EPS = 1e-5
B, D, E, G, H0, H1, H2 = 65536, 256, 8, 128, 256, 128, 64
NCORES = 8
NB = B // NCORES          # rows per core
TB = 512                  # batch tile (matmul free dim / PSUM bank)
NT = NB // TB             # batch tiles per core
KD = D // 128             # k-chunks over D
NPAIR = E // 2

BF16 = ml_dtypes.bfloat16


def _fold_params(inputs):
    """Fold the four BatchNorms into the adjacent Linears. float64 math."""
    f = {k: np.asarray(v, dtype=np.float64) for k, v in inputs.items()}

    s_in = f["in_g"] / np.sqrt(f["in_v"] + EPS)            # [D]
    t_in = f["in_b"] - f["in_m"] * s_in                    # [D]

    # gating L1 (+input BN folded in)
    a_g = f["g_g"] / np.sqrt(f["g_v"] + EPS)               # [G]
    w1 = f["gW1"] * a_g[None, :]                           # [D,G]
    W1f = s_in[:, None] * w1
    b1f = t_in @ w1 + (f["gb1"] - f["g_m"]) * a_g + f["g_b"]

    # expert L0 (+input BN)
    a0 = f["e0g"] / np.sqrt(f["e0v"] + EPS)                # [E,H0]
    w0 = f["eW0"] * a0[:, None, :]                         # [E,D,H0]
    W0f = s_in[None, :, None] * w0
    b0f = np.einsum("d,edo->eo", t_in, w0) + (f["eb0"] - f["e0m"]) * a0 + f["e0b"]

    a1 = f["e1g"] / np.sqrt(f["e1v"] + EPS)
    W1ef = f["eW1"] * a1[:, None, :]                       # [E,H0,H1]
    b1ef = (f["eb1"] - f["e1m"]) * a1 + f["e1b"]

    a2 = f["e2g"] / np.sqrt(f["e2v"] + EPS)
    W2f = f["eW2"] * a2[:, None, :]                        # [E,H1,H2]
    b2f = (f["eb2"] - f["e2m"]) * a2 + f["e2b"]

    g32 = lambda a: np.ascontiguousarray(a, dtype=np.float32)
    gbf = lambda a: np.ascontiguousarray(a.astype(np.float32)).astype(BF16)

    dev = {}
    dev["WG1"] = gbf(W1f.reshape(KD, 128, G).transpose(1, 0, 2))          # [128,KD,G]
    dev["BG1"] = g32(b1f[:, None])                                        # [G,1]
    dev["WE0"] = gbf(W0f.reshape(E, KD, 128, 2, 128).transpose(2, 0, 1, 3, 4))  # [128,E,KD,2,128]
    dev["BE0"] = g32(b0f.reshape(E, 2, 128).transpose(2, 0, 1))           # [128,E,2]
    dev["WE1"] = gbf(W1ef.reshape(E, 2, 128, H1).transpose(2, 0, 1, 3))   # [128,E,2,H1]
    dev["BE1"] = g32(b1ef.T)                                              # [H1,E]
    dev["WE2"] = gbf(W2f.reshape(NPAIR, 2, H1, H2).transpose(2, 0, 1, 3)) # [128,NPAIR,2,64]
    BE2 = np.zeros((128, NPAIR), dtype=np.float64)
    for j in range(NPAIR):
        BE2[0:64, j] = b2f[2 * j]
        BE2[64:128, j] = b2f[2 * j + 1]
    dev["BE2"] = g32(BE2)

    host = {
        "gW2": np.ascontiguousarray(f["gW2"]),     # [G,E] f64
        "gb2": np.ascontiguousarray(f["gb2"]),     # [E]
        "ow": np.ascontiguousarray(f["oW"][:, 0]), # [H2]
        "ob": float(f["ob"][0]),
    }
    return dev, host


def _build_program():
    import concourse.bass as bass
    import concourse.mybir as mybir
    import concourse.tile as tile
    from concourse import bacc

    f32 = mybir.dt.float32
    bf16 = mybir.dt.bfloat16
    Relu = mybir.ActivationFunctionType.Relu
    add = mybir.AluOpType.add
    amax = mybir.AluOpType.max

    nc = bacc.Bacc("TRN2", target_bir_lowering=False, debug=False)

    xT = nc.dram_tensor("xT", [D, NB], bf16, kind="ExternalInput").ap()
    ghout = nc.dram_tensor("ghout", [G, NB], bf16, kind="ExternalOutput").ap()
    h2out = nc.dram_tensor("h2out", [128, NPAIR, NB], bf16, kind="ExternalOutput").ap()
    d_in = {}
    for name, shape, dt in [
        ("WG1", [128, KD, G], bf16), ("BG1", [G, 1], f32),
        ("WE0", [128, E, KD, 2, 128], bf16), ("BE0", [128, E, 2], f32),
        ("WE1", [128, E, 2, H1], bf16), ("BE1", [H1, E], f32),
        ("WE2", [128, NPAIR, 2, H2], bf16), ("BE2", [128, NPAIR], f32),
    ]:
        d_in[name] = nc.dram_tensor(name, shape, dt, kind="ExternalInput").ap()

    def dtype_of(name):
        return bf16 if name.startswith("W") else f32

    with tile.TileContext(nc) as tc:
        with (
            tc.tile_pool(name="consts", bufs=1) as consts,
            tc.tile_pool(name="xt", bufs=3) as xtp,
            tc.tile_pool(name="gh", bufs=3) as ghp,
            tc.tile_pool(name="h0", bufs=10) as h0p,
            tc.tile_pool(name="h1", bufs=6) as h1p,
            tc.tile_pool(name="h2", bufs=6) as h2p,
            tc.tile_pool(name="pmm", bufs=7, space="PSUM") as pmm,
        ):
            W = {}
            for name, ap in d_in.items():
                W[name] = consts.tile(list(ap.shape), dtype_of(name), tag=name, name=name)
                if name in ("WE0", "WE1", "WE2"):
                    continue  # usage-order per-expert loads below
                nc.gpsimd.dma_start(W[name][:], ap[:])
            for e in range(E):
                nc.gpsimd.dma_start(W["WE0"][:, e], d_in["WE0"][:, e])
                nc.gpsimd.dma_start(W["WE1"][:, e], d_in["WE1"][:, e])
                if e < NPAIR:
                    nc.gpsimd.dma_start(W["WE2"][:, e], d_in["WE2"][:, e])

            for t in range(NT):
                bs = t * TB
                xt = xtp.tile([128, KD, TB], bf16, tag="xt")
                for c in range(KD):
                    nc.sync.dma_start(xt[:, c, :], xT[c * 128:(c + 1) * 128, bs:bs + TB])

                # ---- gating L1: gh = relu(W1f^T xn + b1f), exported ----
                psg = pmm.tile([128, TB], f32, tag="mm", name="psg")
                for c in range(KD):
                    nc.tensor.matmul(psg[:], W["WG1"][:, c, :], xt[:, c, :],
                                     start=(c == 0), stop=(c == KD - 1))
                gh = ghp.tile([128, TB], bf16, tag="gh")
                nc.scalar.activation(gh[:], psg[:], Relu, bias=W["BG1"][:, 0:1])
                nc.gpsimd.dma_start(ghout[:, bs:bs + TB], gh[:])

                # ---- expert L0 (all experts): D=256 -> H0=256 ----
                h0 = []
                for e in range(E):
                    ps0 = [pmm.tile([128, TB], f32, tag="mm", name=f"ps0_{e}_{m}")
                           for m in range(2)]
                    for mc in range(2):
                        for c in range(KD):
                            nc.tensor.matmul(ps0[mc][:], W["WE0"][:, e, c, mc, :],
                                             xt[:, c, :],
                                             start=(c == 0), stop=(c == KD - 1))
                    h0e = h0p.tile([128, 2, TB], bf16, tag="h0", name=f"h0_{e}")
                    nc.scalar.activation(h0e[:, 0, :], ps0[0][:], Relu,
                                         bias=W["BE0"][:, e, 0:1])
                    nc.vector.tensor_scalar(out=h0e[:, 1, :], in0=ps0[1][:],
                                            scalar1=W["BE0"][:, e, 1:2], scalar2=0.0,
                                            op0=add, op1=amax)
                    h0.append(h0e)

                # ---- expert L1 (all experts): H0=256 -> H1=128 ----
                h1 = [h1p.tile([128, 2, TB], bf16, tag="h1", name=f"h1_{j}")
                      for j in range(NPAIR)]
                for e in range(E):
                    ps1 = pmm.tile([128, TB], f32, tag="mm", name=f"ps1_{e}")
                    for c in range(2):
                        nc.tensor.matmul(ps1[:], W["WE1"][:, e, c, :], h0[e][:, c, :],
                                         start=(c == 0), stop=(c == 1))
                    j, i = divmod(e, 2)
                    if i == 0:
                        nc.scalar.activation(h1[j][:, 0, :], ps1[:], Relu,
                                             bias=W["BE1"][:, e:e + 1])
                    else:
                        nc.vector.tensor_scalar(out=h1[j][:, 1, :], in0=ps1[:],
                                                scalar1=W["BE1"][:, e:e + 1],
                                                scalar2=0.0, op0=add, op1=amax)

                # ---- expert L2 (per pair, col-tiled concurrent M=64 x2) ----
                for j in range(NPAIR):
                    ps2 = pmm.tile([128, TB], f32, tag="mm", name=f"ps2_{j}")
                    nc.tensor.matmul(ps2[0:64, :], W["WE2"][:, j, 0, :],
                                     h1[j][:, 0, :], start=True, stop=True,
                                     tile_position=(0, 0))
                    nc.tensor.matmul(ps2[64:128, :], W["WE2"][:, j, 1, :],
                                     h1[j][:, 1, :], start=True, stop=True,
                                     tile_position=(0, 64))
                    h2 = h2p.tile([128, TB], bf16, tag="h2", name=f"h2_{j}")
                    if j % 2 == 0:
                        nc.scalar.activation(h2[:], ps2[:], Relu,
                                             bias=W["BE2"][:, j:j + 1])
                    else:
                        nc.vector.tensor_scalar(out=h2[:], in0=ps2[:],
                                                scalar1=W["BE2"][:, j:j + 1],
                                                scalar2=0.0, op0=add, op1=amax)
                    nc.gpsimd.dma_start(h2out[:, j, bs:bs + TB], h2[:])

    nc.compile()
    return nc


_CACHE = {}


def _get_program():
    if "nc" not in _CACHE:
        _CACHE["nc"] = _build_program()
    return _CACHE["nc"]


def _run(inputs, trace=False):
    from concourse.bass_utils import run_bass_kernel_spmd

    x = np.ascontiguousarray(np.asarray(inputs["x"], dtype=np.float32))
    dev, host = _fold_params(inputs)
    nc = _get_program()

    in_maps = []
    for c in range(NCORES):
        m = dict(dev)
        m["xT"] = np.ascontiguousarray(x[c * NB:(c + 1) * NB, :].T).astype(BF16)
        in_maps.append(m)

    kwargs = {}
    if trace:
        kwargs = dict(trace=True, trace_cores=[0])
    res = run_bass_kernel_spmd(nc, in_maps, core_ids=list(range(NCORES)), **kwargs)

    # host tail: gating L2 + softmax weights + s = ow . h2 + combine
    gh = np.concatenate([np.asarray(res.results[c]["ghout"]).astype(np.float32)
                         for c in range(NCORES)], axis=1)          # [G, B]
    h2 = np.concatenate([np.asarray(res.results[c]["h2out"]).astype(np.float32)
                         for c in range(NCORES)], axis=2)          # [128, NPAIR, B]
    logits = gh.T.astype(np.float64) @ host["gW2"] + host["gb2"]   # [B, E]
    expg = np.exp(logits - logits.max(axis=1, keepdims=True))      # [B, E]
    ow = host["ow"].astype(np.float32)
    s_even = np.einsum("k,kjb->jb", ow, h2[0:64], optimize=True)   # [NPAIR, B]
    s_odd = np.einsum("k,kjb->jb", ow, h2[64:128], optimize=True)  # [NPAIR, B]
    s = np.empty((E, B), dtype=np.float64)
    s[0::2] = s_even
    s[1::2] = s_odd
    num = np.einsum("be,eb->b", expg, s)
    den = expg.sum(axis=1)
    out = num / den + host["ob"]
    return out.astype(np.float32)[:, None], res


def kernel(**inputs):
    out, _ = _run(inputs, trace=False)
    return out


def kernel_traced(**inputs):
    return _run(inputs, trace=True)
